# revision 1
# baseline (speedup 1.0000x reference)
"""Bahdanau-style sparse attention kernel for Trainium2, data-parallel over
batch on 8 cores.

Math (per batch row b):
    h_proj = hidden @ Wh.T + b_attn                      [128]
    energy[s, :] = tanh(h_proj + embs[s] @ We.T)         [S, 128]
    att[s] = v . energy[s, :]                            [S]
    out = softmax_S(where(mask==0, -1e10, att))

Sparsity: masked positions (mask==0, ~50%) contribute exactly 0 to the
output, so the host compacts each row to its unmasked columns (gather /
zero-pad to a static FW-multiple width W chosen from the actual mask at
first call; index bookkeeping + data movement only — all FLOPs stay on
device) and scatters the device result back, leaving exact zeros at
masked positions like the reference.

Device strategy per core (8 batch rows, W compacted columns each):
  - One HWDGE DMA ring carries all input transfers in priority order:
    fppack (all fp16 consts) -> b0 pieces -> b1/b2 per-chunk pieces ->
    b3-7 whole rows -> bones (f32, epilogue-only). Compute chases the
    stream; each dma_start costs ~0.6us of sequencer issue and ~2us of
    completion-receipt latency, so the head is few + small.
  - PE: warm-up matmuls on zeros unthrottle HAM during the DMA head;
    We-matmuls produce e_projT in [128, <=1536] PSUM tiles; one-hot-
    column v-matmuls contract d and scatter each (b, FW-chunk) att row
    into a persistent [NP, FW] PSUM accumulator (partition = CPB*b +
    s//FW), pre-seeded with the pad/mask bias via an identity matmul.
  - ACT: tanh/exp table set preloaded on a dummy; tanh with per-
    partition bias h_projT[:, b] in [1536, rest] calls per row; one
    FW-wide exp with accum_out for the softmax partial sums.
  - Epilogue: block-ones matmul turns chunk partials into per-partition
    row denominators in one step; reciprocal + scale; single out DMA.
  - Softmax skips max-subtraction: |att| <= ||v||_1 ~ 5.7 so exp is
    safe; pad bias of -30 keeps padded columns out of the denominator.
"""

import numpy as np

B = 64
S = 4096
D = 128  # dec_dim == emb_dim == 128
NCORES = 8
BPC = B // NCORES  # 8 batch rows per core
FW = 384  # att accumulator chunk width (v-matmul N, <=512 for one bank)
TW = 1536  # e_proj / tanh tile width (3 PSUM banks)

_COMPILED = {}


def _chunks(w, first=None):
    """Split w into FW-multiple pieces <= TW (optionally a given first)."""
    out = []
    if first and w > first:
        out.append(first)
        w -= first
    while w > 0:
        p = min(TW, w)
        out.append(p)
        w -= p
    return out


def _build_bass(W):
    import concourse.bacc as bacc
    import concourse.mybir as mybir
    from concourse.tile import TileContext

    f32 = mybir.dt.float32
    fp16 = mybir.dt.float16
    AF = mybir.ActivationFunctionType

    CPB = W // FW  # FW-chunks per batch row
    NP = BPC * CPB  # att accumulator partitions (<=88 for W<=S)

    nc = bacc.Bacc(
        "TRN2", target_bir_lowering=False, debug=False, num_devices=NCORES
    )

    embsT = nc.dram_tensor("embsT", [BPC, D, W], fp16, kind="ExternalInput")
    # fp16 consts: maskC | I | WeT | vstrip | WhT | hiddenT | b_attn(x8)
    FPW = FW + NP + D + 2 * NP + D + BPC + BPC
    fppack = nc.dram_tensor("fppack", [D, FPW], fp16, kind="ExternalInput")
    # f32 epilogue consts: block-ones [NP, NP]
    bones = nc.dram_tensor("bones", [NP, NP], f32, kind="ExternalInput")
    out_d = nc.dram_tensor("out", [NP, FW], f32, kind="ExternalOutput")

    with TileContext(nc) as tc:
        with (
            tc.tile_pool(name="consts", bufs=1) as consts,
            tc.tile_pool(name="embs0", bufs=1) as embs0_pool,
            tc.tile_pool(name="embsab", bufs=2) as embsab_pool,
            tc.tile_pool(name="embs16", bufs=5) as embs16_pool,
            tc.tile_pool(name="energy", bufs=7) as energy_pool,
            tc.tile_pool(name="post", bufs=1) as post,
            tc.tile_pool(name="ps_big", bufs=2, space="PSUM") as ps_big,
            tc.tile_pool(name="ps_att", bufs=1, space="PSUM") as ps_att,
        ):
            # --- input DMAs, one ring, priority order ---
            fppack_sb = consts.tile([D, FPW], fp16)
            nc.sync.dma_start(out=fppack_sb, in_=fppack[:, :])
            o = 0
            maskC_sb = fppack_sb[0:NP, o : o + FW]; o += FW
            idNP_sb = fppack_sb[0:NP, o : o + NP]; o += NP
            WeT_sb = fppack_sb[:, o : o + D]; o += D
            vstrip_sb = fppack_sb[:, o : o + 2 * NP]; o += 2 * NP
            WhT_sb = fppack_sb[:, o : o + D]; o += D
            hiddenT_sb = fppack_sb[:, o : o + BPC]; o += BPC
            b_attn_sb = fppack_sb[:, o : o + BPC]

            b0_chunks = _chunks(W, first=2 * FW)
            b_chunks = _chunks(W)
            # b0: first piece alone (fast first tanh), remainder as one DMA
            QW = b0_chunks[0]
            et0a = embs0_pool.tile([D, QW], fp16, tag="et0a")
            nc.sync.dma_start(out=et0a, in_=embsT[0, :, 0:QW])
            et0b = None
            if W > QW:
                et0b = embs0_pool.tile([D, W - QW], fp16, tag="et0b")
                nc.sync.dma_start(out=et0b, in_=embsT[0, :, QW:W])
            # b1/b2: per-chunk pieces (finer deps while the stream ramps)
            ets = [None]
            for b in range(1, BPC):
                if b <= 2 and len(b_chunks) > 1:
                    pieces = []
                    off = 0
                    for w in b_chunks:
                        t = embsab_pool.tile([D, w], fp16, tag=f"e{b}_{off}")
                        nc.sync.dma_start(out=t, in_=embsT[b, :, off : off + w])
                        pieces.append((off, w, t))
                        off += w
                    ets.append(pieces)
                else:
                    et = embs16_pool.tile([D, W], fp16, tag="et")
                    nc.sync.dma_start(out=et, in_=embsT[b, :, :])
                    ets.append(et)
            bones_sb = consts.tile([NP, NP], f32)
            nc.sync.dma_start(out=bones_sb, in_=bones[:, :])

            # ACT table preload on a dummy (off the critical path)
            dummy = consts.tile([1, 8], f32)
            nc.vector.memset(dummy[:, :], 0.0)
            dummy2 = consts.tile([1, 8], f32)
            nc.scalar.activation(out=dummy2[:, :], in_=dummy[:, :], func=AF.Tanh)

            # PE warm-up on zero tiles: >3.4us of matmuls during the DMA
            # head so HAM unthrottles to 2.4GHz before the real matmuls.
            wu_w = consts.tile([D, D], fp16)
            nc.vector.memset(wu_w[:, :], 0.0)
            wu_m = consts.tile([D, FW], fp16)
            nc.vector.memset(wu_m[:, :], 0.0)
            for _ in range(8):
                wu_ps = ps_big.tile([D, TW], f32, tag="ps")
                nc.tensor.matmul(wu_ps[:, 0:FW], wu_w[:, :], wu_m[:, :])

            # h_projT[d, b] = sum_k WhT[k, d] * hiddenT[k, b] + b_attn[d]
            hp_ps = ps_big.tile([D, BPC], f32, tag="ps")
            nc.tensor.matmul(hp_ps[:, :], WhT_sb[:, :], hiddenT_sb[:, :])
            hprojT_sb = consts.tile([D, BPC], f32)
            nc.vector.tensor_add(hprojT_sb[:, :], hp_ps[:, :], b_attn_sb[:, :])

            # att accumulator [NP, FW]: partition CPB*b + s//FW, free s%FW,
            # seeded with the pad/mask bias (0/-30) via I @ maskC.
            att_ps = ps_att.tile([NP, FW], f32)
            n_mm_tot = 1 + BPC * CPB
            n_vmm = 0

            def emit_mask_mm():
                nonlocal n_vmm
                nc.tensor.matmul(
                    att_ps[:, :],
                    idNP_sb[:, :],
                    maskC_sb[:, :],
                    start=True,
                    stop=False,
                    skip_group_check=True,
                )
                n_vmm += 1

            def emit_vmms(pending):
                nonlocal n_vmm
                for en_t, b, off, w in pending:
                    for m in range(w // FW):
                        p = CPB * b + (off + m * FW) // FW
                        nc.tensor.matmul(
                            att_ps[:, :],
                            vstrip_sb[:, NP - p : 2 * NP - p],
                            en_t[:, m * FW : (m + 1) * FW],
                            start=False,
                            stop=(n_vmm >= n_mm_tot - 1),
                            skip_group_check=True,
                        )
                        n_vmm += 1

            def do_tile(b, off, w, src, src_off):
                pe_t = ps_big.tile([D, TW], f32, tag="ps")
                # e_proj matmuls in 512-wide pieces (PSUM-bank aligned),
                # independent of the FW-wide v-matmul chunking.
                mo = 0
                while mo < w:
                    mw = min(512, w - mo)
                    nc.tensor.matmul(
                        pe_t[:, mo : mo + mw],
                        WeT_sb[:, :],
                        src[:, src_off + mo : src_off + mo + mw],
                    )
                    mo += mw
                en_t = energy_pool.tile([D, TW], fp16)
                nc.scalar.activation(
                    out=en_t[:, 0:w],
                    in_=pe_t[:, 0:w],
                    func=AF.Tanh,
                    bias=hprojT_sb[:, b : b + 1],
                    scale=1.0,
                )
                return (en_t, b, off, w)

            prev = []
            for b in range(BPC):
                cur = []
                if b == 0:
                    emit_mask_mm()
                    off = 0
                    for i, w in enumerate(b0_chunks):
                        src = et0a if i == 0 else et0b
                        src_off = off if i == 0 else off - QW
                        cur.append(do_tile(0, off, w, src, src_off))
                        off += w
                elif isinstance(ets[b], list):
                    for off, w, t in ets[b]:
                        cur.append(do_tile(b, off, w, t, 0))
                else:
                    off = 0
                    for w in b_chunks:
                        cur.append(do_tile(b, off, w, ets[b], off))
                        off += w
                emit_vmms(prev)
                prev = cur
            emit_vmms(prev)

            # softmax: p = exp(att) with per-partition (chunk) partial sums
            p_sb = post.tile([NP, FW], f32)
            partials_sb = post.tile([NP, 1], f32)
            nc.scalar.activation(
                out=p_sb[:, :],
                in_=att_ps[:, :],
                func=AF.Exp,
                accum_out=partials_sb[:, 0:1],
            )
            # block-ones matmul: den[p] = sum of partials over p's batch row
            den_ps = ps_big.tile([NP, 1], f32, tag="ps")
            nc.tensor.matmul(den_ps[:, :], bones_sb[:, :], partials_sb[:, 0:1])
            recip_sb = post.tile([NP, 1], f32)
            nc.vector.reciprocal(recip_sb[:, :], den_ps[:, :])

            out_sb = post.tile([NP, FW], f32)
            nc.vector.tensor_scalar_mul(out_sb[:, :], p_sb[:, :], recip_sb[:, 0:1])
            nc.sync.dma_start(out=out_d[:, :], in_=out_sb[:, :])

    nc.compile()
    return nc


def _get_nc(W):
    if W not in _COMPILED:
        _COMPILED[W] = _build_bass(W)
    return _COMPILED[W]


def _prep(hidden, seq_embs, mask, W_attn, b_attn, v_w):
    """Host-side prep: batch shard, mask compaction (gather), fp16 cast,
    relayouts. All FLOPs on the model data happen on device."""
    hidden = np.asarray(hidden, dtype=np.float32)
    seq_embs = np.asarray(seq_embs, dtype=np.float32)
    mask = np.asarray(mask)
    W_attn = np.asarray(W_attn, dtype=np.float32)
    b_attn = np.asarray(b_attn, dtype=np.float32)
    v_w = np.asarray(v_w, dtype=np.float32)

    idxs = [np.flatnonzero(mask[g]) for g in range(B)]
    maxc = max((len(ix) for ix in idxs), default=0)
    W = max(FW, -(-maxc // FW) * FW)  # FW-multiple (may exceed S: pad-only)
    CPB = W // FW
    NP = BPC * CPB

    WhT = np.ascontiguousarray(W_attn[:, :D].T)
    WeT = np.ascontiguousarray(W_attn[:, D:].T.astype(np.float16))
    b_col = np.ascontiguousarray(b_attn.reshape(D, 1))
    vstrip = np.zeros((D, 2 * NP), dtype=np.float16)
    vstrip[:, NP] = v_w[0].astype(np.float16)
    # block-ones: den_spread[p] = sum_{p' in same batch row} partials[p']
    bones = np.zeros((NP, NP), dtype=np.float32)
    for p in range(NP):
        b = p // CPB
        bones[b * CPB : (b + 1) * CPB, p] = 1.0

    FPW = FW + NP + D + 2 * NP + D + BPC + BPC

    seq16 = seq_embs.astype(np.float16)  # [S, B, D]
    in_maps = []
    for c in range(NCORES):
        embsT = np.zeros((BPC, D, W), dtype=np.float16)
        maskC = np.full((NP, FW), -30.0, dtype=np.float16)
        for b in range(BPC):
            g = c * BPC + b
            ix = idxs[g]
            cnt = len(ix)
            if cnt:
                embsT[b, :, :cnt] = seq16[ix, g, :].T
                flat = maskC.reshape(-1)
                flat[b * W : b * W + cnt] = 0.0
        bsl = slice(c * BPC, (c + 1) * BPC)
        hiddenT = np.ascontiguousarray(hidden[bsl].T)
        fppack = np.zeros((D, FPW), dtype=np.float16)
        o = 0
        fppack[:NP, o : o + FW] = maskC; o += FW
        fppack[:NP, o : o + NP] = np.eye(NP, dtype=np.float16); o += NP
        fppack[:, o : o + D] = WeT; o += D
        fppack[:, o : o + 2 * NP] = vstrip; o += 2 * NP
        fppack[:, o : o + D] = WhT.astype(np.float16); o += D
        fppack[:, o : o + BPC] = hiddenT.astype(np.float16); o += BPC
        fppack[:, o : o + BPC] = np.repeat(b_col.astype(np.float16), BPC, axis=1)
        in_maps.append({"embsT": embsT, "fppack": fppack, "bones": bones})
    return W, idxs, in_maps


def kernel(hidden, seq_embs, mask, W_attn, b_attn, v_w, **run_kwargs):
    from concourse.bass_utils import run_bass_kernel_spmd

    W, idxs, in_maps = _prep(hidden, seq_embs, mask, W_attn, b_attn, v_w)
    nc = _get_nc(W)
    res = run_bass_kernel_spmd(
        nc, in_maps, core_ids=list(range(NCORES)), **run_kwargs
    )
    out = np.zeros((B, S), dtype=np.float32)
    for c in range(NCORES):
        comp = res.results[c]["out"].reshape(BPC, W).astype(np.float32)
        for b in range(BPC):
            g = c * BPC + b
            ix = idxs[g]
            if len(ix):
                out[g, ix] = comp[b, : len(ix)]
            else:
                out[g, :] = 1.0 / S  # softmax of all -1e10 is uniform
    if run_kwargs:
        kernel.last_results = res  # stash for the profiling harness
    return out

